# revision 1
# baseline (speedup 1.0000x reference)
"""Trainium2 kernel for nn_BasicWHVILinear.

Math (reference):
    qf    = tril(Q) + tril(Q)^T - diag(diag(Q))        (symmetric, 2048x2048)
    Sigma = qf @ qf^T
    L     = cholesky(Sigma)
    g     = q_mu + L @ eps
    u     = H^T @ (s1 * g)                              (H = scaled Hadamard)
    W     = s2[:,None] * H^T * u[None,:]
    out   = relu(x @ W^T),  x: (16384, 2048)

Sharding strategy (per spec hint): data-parallel on the batch axis — the
16384-row x is split into 8 shards of 2048 rows, one per NeuronCore; the
D-dim parameter pipeline (Sigma -> Cholesky -> g -> u -> W, ~7% of total
FLOPs, serial) is replicated preprocessing shared by every shard, and each
core runs the batched GEMM out_c = relu(x_c @ W^T) on device.

Device GEMM design notes (constraints of this walrus/bass toolchain):
  - PE Matmult and SP-issued HWDGE DMACopy instructions only support ONE
    semaphore wait each; walrus codegen hard-fails otherwise. Therefore:
      * every DMA lands in a write-once SBUF destination (no staging rings),
        so no DMA ever needs a prior-writer/reader wait on top of its own
        queue wait;
      * both GEMM operands live fully resident in SBUF in bf16 (8 MB + 8 MB),
        with a DVE self-copy "fence" over each DMA'd region so that every PE
        matmul depends only on the single DVE semaphore;
      * PSUM eviction (fused relu) also runs on DVE, keeping the
        start-of-accumulation matmuls single-wait as well.
  - bf16 operands at fp32 PSUM accumulation, with the output also emitted
    as bf16 and upcast to the fp32 contract on the host: 3.9e-3 relative
    error vs the fp64 oracle (validated off-line), ~5x inside the accuracy
    budget, and the writeback DMA traffic is halved.
  - x^T is pre-transposed and pre-cast on the host so both operands stream
    K-major; 16 MB in + 8 MB out per core sits well under the PE time
    (~224 us).
"""

import os
import numpy as np

D = 2048
BATCH = 16384
N_CORES = 8
ROWS = BATCH // N_CORES  # rows of x per core

P = 128
KT = D // P          # 16 contraction tiles
NQ = 512             # psum free dim (one bank)
NT = D // NQ         # 4 n-chunks
MT = ROWS // P       # 16 output row tiles per core
MCH = 512            # m-chunk for x loads
MCT = ROWS // MCH    # 4 m-chunks

TRACE = bool(int(os.environ.get("WHVI_KERNEL_TRACE", "0")))
LAST_EXEC_TIME_NS = None
LAST_RESULT = None

_PROGRAM = None


def _build_H():
    H = np.array([[1.0, 1.0], [1.0, -1.0]], dtype=np.float32)
    while H.shape[0] < D:
        H = np.block([[H, H], [H, -H]])
    return H * np.float32(D ** -0.5)


def _host_wt(s1, s2, q_mu, q_factor_lower, eps):
    """Replicated parameter pipeline -> W^T (K x N layout for the GEMM)."""
    ql = np.asarray(q_factor_lower, np.float32)
    qf = ql + ql.T - np.diag(np.diag(ql))
    Sigma = qf @ qf.T
    L = np.linalg.cholesky(Sigma)
    g = np.asarray(q_mu, np.float32) + L @ np.asarray(eps, np.float32)
    H = _build_H()
    u = H.T @ (np.asarray(s1, np.float32) * g)
    # W[i, j] = s2[i] * H[j, i] * u[j]  =>  W^T[j, i] = u[j] * H[j, i] * s2[i]
    WT = u[:, None] * H * np.asarray(s2, np.float32)[None, :]
    return np.ascontiguousarray(WT, dtype=np.float32)


def _build_program():
    from contextlib import ExitStack

    import concourse.bacc as bacc
    import concourse.mybir as mybir
    import concourse.tile as tile

    f32 = mybir.dt.float32
    bf16 = mybir.dt.bfloat16

    # Bacc (not raw Bass): its finalize() runs generate_event_semaphores /
    # fuse_nops, which split multi-semaphore waits into EventSemaphore
    # instructions — this walrus only accepts ONE wait per instruction.
    nc = bacc.Bacc()
    xT = nc.declare_dram_parameter("xT", [D, ROWS], bf16, isOutput=False)
    wt = nc.declare_dram_parameter("wt", [D, D], bf16, isOutput=False)
    out = nc.declare_dram_parameter("out", [ROWS, D], bf16, isOutput=True)

    with tile.TileContext(nc) as tc:
        with ExitStack() as ctx:
            big_pool = ctx.enter_context(tc.tile_pool(name="big", bufs=1))
            out_pool = ctx.enter_context(tc.tile_pool(name="outs", bufs=2))
            psum_pool = ctx.enter_context(
                tc.tile_pool(name="psum", bufs=2, space="PSUM")
            )

            # Write-once resident operands.
            wtf = big_pool.tile([P, KT, NT, NQ], bf16)   # 8 MB
            xtf = big_pool.tile([P, KT, ROWS], bf16)     # 8 MB

            wt_v = wt[:].rearrange("(kt p) (n nq) -> p kt n nq", p=P, nq=NQ)
            xT_v = xT[:].rearrange("(kt p) m -> p kt m", p=P)

            # Only 8 physical HWDGE queues exist and queue assignment is
            # global round-robin; a 9th DMA wraps onto a used queue and picks
            # up a ring wait that walrus cannot encode next to a real dep.
            # Budget: 2 wt DMAs + 2 x chunks + 4 out DMAs = exactly 8.
            # The first compute slice (wt n=0, x m-cols 0:512) loads via small
            # DMAs so m=0 matmuls start ~10us in instead of ~40us.
            # First-slice loads on two different engines so the inline
            # DIRECT2D transfers overlap instead of serializing on SP.
            nc.sync.dma_start(wtf[:, :, 0, :], wt_v[:, :, 0, :])
            nc.sync.dma_start(xtf[:, :, 0:512], xT_v[:, :, 0:512])
            nc.sync.dma_start(wtf[:, :, 1:, :], wt_v[:, :, 1:, :])
            nc.sync.dma_start(xtf[:, :, 512:], xT_v[:, :, 512:])
            # DVE fences, first-compute slices first.
            nc.vector.tensor_copy(wtf[:, :, 0, :], wtf[:, :, 0, :])
            nc.vector.tensor_copy(xtf[:, :, 0:512], xtf[:, :, 0:512])
            for n in range(1, NT):
                nc.vector.tensor_copy(wtf[:, :, n, :], wtf[:, :, n, :])
            nc.vector.tensor_copy(xtf[:, :, 512:], xtf[:, :, 512:])

            # out is written back in 4 big DMAs (4 m-tiles each) on the
            # scalar engine — with the 4 input DMAs that is exactly the 8
            # physical HWDGE queues, so no DMA needs a queue-ring wait on
            # top of its DVE dep.
            CHUNKS = [4, 4, 4, 2, 2]
            mbase = 0
            for mb in CHUNKS:
                ot = out_pool.tile([P, 4, D], bf16, tag="ot", name="ot")
                for mloc in range(mb):
                    m = mbase + mloc
                    msl = slice(m * P, (m + 1) * P)
                    psums = [
                        psum_pool.tile([P, NQ], f32, tag=f"ps{n}", name=f"ps{n}")
                        for n in range(NT)
                    ]
                    for k in range(KT):
                        for n in range(NT):
                            nc.tensor.matmul(
                                psums[n][:],
                                xtf[:, k, msl],
                                wtf[:, k, n, :],
                                start=(k == 0),
                                stop=(k == KT - 1),
                            )
                    for n in range(NT):
                        nc.vector.tensor_scalar_max(
                            ot[:, mloc, n * NQ : (n + 1) * NQ], psums[n][:], 0.0
                        )
                out_rows = out[mbase * P : (mbase + mb) * P, :]
                nc.scalar.dma_start(
                    out_rows.rearrange("(mt p) n -> p mt n", p=P), ot[:, :mb, :]
                )
                mbase += mb
    nc.finalize()
    return nc


def kernel(x, s1, s2, q_mu, q_factor_lower, eps):
    global _PROGRAM, LAST_EXEC_TIME_NS, LAST_RESULT
    import ml_dtypes
    from concourse.bass_utils import run_bass_kernel_spmd

    bf16 = ml_dtypes.bfloat16
    x = np.asarray(x, np.float32)
    WT = _host_wt(s1, s2, q_mu, q_factor_lower, eps).astype(bf16)

    if _PROGRAM is None:
        _PROGRAM = _build_program()

    core_ids = list(range(N_CORES))
    in_maps = [
        {
            "xT": np.ascontiguousarray(x[c * ROWS : (c + 1) * ROWS].T.astype(bf16)),
            "wt": WT,
        }
        for c in core_ids
    ]
    res = run_bass_kernel_spmd(_PROGRAM, in_maps, core_ids, trace=TRACE)
    LAST_RESULT = res
    LAST_EXEC_TIME_NS = res.exec_time_ns
    out = np.concatenate(
        [np.asarray(res.results[c]["out"]) for c in core_ids], axis=0
    )
    # device emits bf16 (halves the writeback DMA); upcast to the fp32 contract
    return np.ascontiguousarray(out.astype(np.float32))



# revision 5
# speedup vs baseline: 1.2021x; 1.2021x over previous
"""Trainium2 kernel for nn_BasicWHVILinear — Hadamard-factorized version.

Math (reference):
    qf    = tril(Q) + tril(Q)^T - diag(diag(Q))        (symmetric, 2048x2048)
    Sigma = qf @ qf^T ; L = cholesky(Sigma) ; g = q_mu + L @ eps
    u     = H^T @ (s1 * g)                              (H = scaled Hadamard)
    W     = s2[:,None] * H^T * u[None,:]
    out   = relu(x @ W^T),  x: (16384, 2048)

Key algebraic rewrite: out[b,i] = relu(s2[i] * sum_j x[b,j] u[j] H[j,i]).
With the Sylvester Kronecker split H_2048 = H_8 (x) H_256 (j = jA*256+jB,
i = iA*256+iB, H[j,i] = H8[jA,iA] * H256[jB,iB]):
    t[b,jA,iB] = sum_jB x[b,jA*256+jB] * u[..] * H256s[jB,iB]   (PE, K=256)
    y[b,iA,iB] = FWHT_8 over jA of t                            (3 +/- stages)
    out        = relu(s2 * y)                                    (fused max/mult)
This cuts PE work 8x vs the dense GEMM (2*2048^3 -> 2*2048^2*256 flops/core)
and moves the tiny H_8 transform to the vector engines as pure add/sub
butterflies (coefficients exactly +-1; all scaling folded into H256s).

Sharding: data-parallel on batch (8 cores x 2048 rows), parameter pipeline
(Sigma -> cholesky -> u, plus hq/s2 operand packing) replicated on host.

Toolchain constraints honored (see dense baseline for background):
  - PE matmuls and SP-issued DMAs accept ONE semaphore wait. All matmul
    upstream deps (operand DMA fences + psum-bank recycling) funnel through
    the DVE semaphore: fences are DVE self-copies, and the psum reader (b1)
    is a DVE op. Out-DMAs wait only on the DVE fused relu*s2 op.
  - 8 HWDGE queues, global round-robin: consts 1 + x 3 + out 4 = 8 DMAs.
  - bf16 everywhere off-psum (7.6e-3 rel err vs fp64 oracle, validated).
Butterflies alternate DVE (b1) and GpSimd (b2, b3) so the two vector
engines pipeline; relu*s2 is one fused scalar_tensor_tensor on DVE.
"""

import os
import numpy as np

D = 2048
BATCH = 16384
N_CORES = 8
ROWS = BATCH // N_CORES  # 2048 rows of x per core

P = 128
JA = 8                   # H_8 factor (butterfly axis)
NB = 256                 # H_256 factor (PE contraction axis)
KB = NB // P             # 2 contraction subtiles of 128
NCH = 16                 # 2048 rows / 128-row chunks
GRP = 4                  # output chunks per writeback DMA

TRACE = bool(int(os.environ.get("WHVI_KERNEL_TRACE", "0")))
LAST_EXEC_TIME_NS = None
LAST_RESULT = None

_PROGRAM = None


def _had(n):
    H = np.array([[1.0]], dtype=np.float64)
    while H.shape[0] < n:
        H = np.block([[H, H], [H, -H]])
    return H


def _host_operands(s1, s2, q_mu, q_factor_lower, eps):
    """Replicated D-dim parameter pipeline -> (hq, s2rep) device operands.

    hq[p, ja, kb, ib] = u[ja*256 + kb*128 + p] * H256[kb*128+p, ib] * D^-1/2
    s2rep[p, i]       = s2[i]   (broadcast over partitions)
    """
    ql = np.asarray(q_factor_lower, np.float64)
    qf = ql + ql.T - np.diag(np.diag(ql))
    L = np.linalg.cholesky(qf @ qf.T)
    g = np.asarray(q_mu, np.float64) + L @ np.asarray(eps, np.float64)
    Hs = _had(D) * (D ** -0.5)
    u = Hs.T @ (np.asarray(s1, np.float64) * g)                 # (D,)
    Hb_s = _had(NB) * (D ** -0.5)                               # scale on H256
    hq = u.reshape(JA, KB, P)[..., None] * Hb_s.reshape(KB, P, NB)[None]
    hq = np.ascontiguousarray(hq.transpose(2, 0, 1, 3))         # [p, ja, kb, ib]
    s2rep = np.broadcast_to(np.asarray(s2, np.float32), (P, D))
    return hq.astype(np.float32), np.ascontiguousarray(s2rep)


def _build_program():
    from contextlib import ExitStack

    import concourse.bacc as bacc
    import concourse.mybir as mybir
    import concourse.tile as tile

    f32 = mybir.dt.float32
    bf16 = mybir.dt.bfloat16
    alu = mybir.AluOpType

    nc = bacc.Bacc()
    xT = nc.declare_dram_parameter("xT", [D, ROWS], bf16, isOutput=False)
    # hq [P, JA*KB*NB] followed by s2rep [P, D], packed to share one DMA
    consts = nc.declare_dram_parameter("consts", [P, JA * KB * NB + D], bf16,
                                       isOutput=False)
    out = nc.declare_dram_parameter("out", [ROWS, D], bf16, isOutput=True)

    with tile.TileContext(nc) as tc:
        with ExitStack() as ctx:
            big_pool = ctx.enter_context(tc.tile_pool(name="big", bufs=1))
            ping_pool = ctx.enter_context(tc.tile_pool(name="ping", bufs=2))
            pong_pool = ctx.enter_context(tc.tile_pool(name="pong", bufs=2))
            ev_pool = ctx.enter_context(tc.tile_pool(name="ev", bufs=2))
            z_pool = ctx.enter_context(tc.tile_pool(name="z", bufs=3))
            psumA_pool = ctx.enter_context(
                tc.tile_pool(name="psumA", bufs=2, space="PSUM")
            )
            psumB_pool = ctx.enter_context(
                tc.tile_pool(name="psumB", bufs=2, space="PSUM")
            )

            xtf = big_pool.tile([P, JA, KB, ROWS], bf16)   # 8 MB resident x^T
            ct = big_pool.tile([P, JA * KB * NB + D], bf16)
            hq = ct[:, : JA * KB * NB].rearrange(
                "p (ja kb ib) -> p ja kb ib", ja=JA, kb=KB
            )
            s2t = ct[:, JA * KB * NB:]

            xT_v = xT[:].rearrange("(ja kb p) b -> p ja kb b", p=P, kb=KB)

            # 8 DMA budget: consts + 3 x slices + 4 out groups.
            nc.sync.dma_start(ct[:, :], consts[:, :])
            nc.sync.dma_start(xtf[:, :, :, 0:512], xT_v[:, :, :, 0:512])
            nc.sync.dma_start(xtf[:, :, :, 512:1280], xT_v[:, :, :, 512:1280])
            nc.sync.dma_start(xtf[:, :, :, 1280:2048], xT_v[:, :, :, 1280:2048])

            # DVE fences: collapse DMA-queue deps into the DVE semaphore so
            # every matmul needs only its single DVE wait. Ordered so later
            # x-slice fences do not head-of-line-block early butterfly work.
            nc.vector.tensor_copy(ct[:, :], ct[:, :])
            nc.vector.tensor_copy(xtf[:, :, :, 0:512], xtf[:, :, :, 0:512])

            def b1(c, ptA, ptB, ev, png):
                # butterfly dist 4. Tensor ops may read at most ONE psum
                # operand, so Scalar first evicts the jA=4..7 half to SBUF;
                # DVE's add/sub then each read one psum + one sbuf input.
                # The split psum tiles also keep matmuls single-wait: ptA
                # recycling is gated by DVE, ptB by Scalar.
                nc.scalar.activation(
                    ev[:, :, :], ptB[:, :, :],
                    mybir.ActivationFunctionType.Copy)
                nc.vector.tensor_tensor(
                    png[:, 0:4, :], ptA[:, :, :], ev[:, :, :], alu.add)
                nc.vector.tensor_tensor(
                    png[:, 4:8, :], ptA[:, :, :], ev[:, :, :], alu.subtract)

            def b2b3(c, png, pog):
                # dist 2 then dist 1, both on GpSimd; b3 writes back into ping
                v_in = png.rearrange("p (g a) ib -> p g a ib", g=2)
                v_out = pog.rearrange("p (g a) ib -> p g a ib", g=2)
                nc.gpsimd.tensor_tensor(
                    v_out[:, :, 0:2, :], v_in[:, :, 0:2, :], v_in[:, :, 2:4, :],
                    alu.add)
                nc.gpsimd.tensor_tensor(
                    v_out[:, :, 2:4, :], v_in[:, :, 0:2, :], v_in[:, :, 2:4, :],
                    alu.subtract)
                w_in = pog.rearrange("p (g a) ib -> p g a ib", g=4)
                w_out = png.rearrange("p (g a) ib -> p g a ib", g=4)
                nc.gpsimd.tensor_tensor(
                    w_out[:, :, 0:1, :], w_in[:, :, 0:1, :], w_in[:, :, 1:2, :],
                    alu.add)
                nc.gpsimd.tensor_tensor(
                    w_out[:, :, 1:2, :], w_in[:, :, 0:1, :], w_in[:, :, 1:2, :],
                    alu.subtract)

            def fused(c, png, zt):
                # out = max(y, 0) * s2  in one DVE pass, bf16 into staging
                nc.vector.scalar_tensor_tensor(
                    zt[:, c % GRP, :],
                    png.rearrange("p a ib -> p (a ib)"),
                    0.0, s2t, alu.max, alu.mult)

            pings = {}
            zts = {}
            prev = None  # (c, ping, zt) pending fused pass
            for c in range(NCH):
                csl = slice(c * P, (c + 1) * P)
                # late fences gate the x slices only where first needed
                if c == 4:
                    nc.vector.tensor_copy(
                        xtf[:, :, :, 512:1280], xtf[:, :, :, 512:1280])
                if c == 10:
                    nc.vector.tensor_copy(
                        xtf[:, :, :, 1280:2048], xtf[:, :, :, 1280:2048])

                ptA = psumA_pool.tile([P, 4, NB], f32, tag="ptA", name="ptA")
                ptB = psumB_pool.tile([P, 4, NB], f32, tag="ptB", name="ptB")
                for ja in range(JA):
                    dst = ptA[:, ja, :] if ja < 4 else ptB[:, ja - 4, :]
                    for kb in range(KB):
                        nc.tensor.matmul(
                            dst,
                            xtf[:, ja, kb, csl],
                            hq[:, ja, kb, :],
                            start=(kb == 0),
                            stop=(kb == KB - 1),
                        )
                png = ping_pool.tile([P, JA, NB], bf16, tag="ping", name="ping")
                pog = pong_pool.tile([P, JA, NB], bf16, tag="pong", name="pong")
                ev = ev_pool.tile([P, 4, NB], bf16, tag="ev", name="ev")
                if c % GRP == 0:
                    zts[c // GRP] = z_pool.tile([P, GRP, D], bf16, tag="z",
                                                name="z")
                b1(c, ptA, ptB, ev, png)
                b2b3(c, png, pog)
                pings[c] = png
                # software pipeline: issue fused(c-1) after b1(c) so the DVE
                # never head-of-line blocks on GpSimd's butterfly tail
                if prev is not None:
                    fused(*prev)
                    pc = prev[0]
                    if pc % GRP == GRP - 1:
                        d = pc // GRP
                        rows = out[d * GRP * P: (d + 1) * GRP * P, :]
                        nc.scalar.dma_start(
                            rows.rearrange("(q p) n -> p q n", p=P),
                            zts[d][:, :, :])
                prev = (c, png, zts[c // GRP])
            fused(*prev)
            d = NCH // GRP - 1
            rows = out[d * GRP * P: (d + 1) * GRP * P, :]
            nc.scalar.dma_start(
                rows.rearrange("(q p) n -> p q n", p=P), zts[d][:, :, :])
    nc.finalize()
    return nc


def kernel(x, s1, s2, q_mu, q_factor_lower, eps):
    global _PROGRAM, LAST_EXEC_TIME_NS, LAST_RESULT
    import ml_dtypes
    from concourse.bass_utils import run_bass_kernel_spmd

    bf16 = ml_dtypes.bfloat16
    x = np.asarray(x, np.float32)
    hq, s2rep = _host_operands(s1, s2, q_mu, q_factor_lower, eps)
    consts = np.concatenate(
        [hq.reshape(P, JA * KB * NB), s2rep], axis=1).astype(bf16)

    if _PROGRAM is None:
        _PROGRAM = _build_program()

    core_ids = list(range(N_CORES))
    in_maps = [
        {
            "xT": np.ascontiguousarray(x[c * ROWS: (c + 1) * ROWS].T.astype(bf16)),
            "consts": consts,
        }
        for c in core_ids
    ]
    res = run_bass_kernel_spmd(_PROGRAM, in_maps, core_ids, trace=TRACE)
    LAST_RESULT = res
    LAST_EXEC_TIME_NS = res.exec_time_ns
    out = np.concatenate(
        [np.asarray(res.results[c]["out"]) for c in core_ids], axis=0
    )
    return np.ascontiguousarray(out.astype(np.float32))
